# revision 19
# baseline (speedup 1.0000x reference)
"""Trainium2 Bass kernel for nn_Attention (dense transformer attention block).

Computation (per reference):
  q = x @ wq.T; k = x @ wk.T; v = x @ wv.T       (GQA: 16 q heads, 4 kv heads)
  rope(q, k) with cos/sin from freqs (interleaved complex pairs)
  non-causal SDPA with softmax over keys, scale 1/sqrt(128)
  out = (probs @ v reshaped) @ wo.T

Sharding (8 cores): tensor-parallel over the 4 kv-head groups (TP=4; each
core gets 4 q heads + 1 kv head, wq/wk/wv column-sharded, wo row-sharded)
x data-parallel over batch (DP=2; 2 batches per core). Each core computes a
partial output [2, S, DIM]; the host sums the 4 TP partials per batch pair.

Device layout notes:
 - x is passed transposed per batch: xt[b] = x[b].T  [DIM, S] (bf16) so that
   projections contract over DIM on the partition axis.
 - All matmul operands are bf16 (fp32 PSUM accumulation): enables fast
   weight load and halves HBM traffic; measured end-to-end error ~7e-3.
 - Q^T/K^T are computed in [head_dim, S] layout; per head the 128
   head-dim rows are permuted to [evens(64) | odds(64)] (done by permuting
   wq/wk rows on the host) so RoPE pairs are partition-contiguous halves.
   RoPE runs per head as soon as that head's accumulation finishes
   (head-major matmul order) so PSUM slots recycle without stalling PE.
 - scores are computed transposed: S^T[k, q] = K^T.T @ Q^T per 128k x 512q
   tile; exp via ACT (scale folded in). Phase B is software-pipelined: the
   AV / denominator matmuls for score-group kg are emitted one group late
   so the in-order PE queue never stalls on the ACT exp.
 - softmax denominators: 4 col-tiled ones-matmuls (M=32 broadcast rows,
   emitted adjacently so they pack into the PE array concurrently)
   accumulate per-column partial sums; a selection-matrix matmul sums the
   4 partials and broadcasts to 128 partitions; reciprocal on DVE; the AV
   output is scaled by it.
 - PSUM plan (8 banks): tag "pair" = 3 x [128,2,512] rotating slots
   (projection q pairs / score groups / phase C), tag "av" = 1 dedicated
   slot (projection k+v / AV+colsum accumulator).
 - AV output is produced in [head_dim, S] layout which directly feeds the
   wo matmul as lhsT.
"""

import numpy as np
from contextlib import ExitStack

import ml_dtypes

import concourse.bacc as bacc
import concourse.tile as tile
from concourse import mybir
from concourse.bass_utils import run_bass_kernel_spmd
from concourse.masks import make_identity

F32 = mybir.dt.float32
F32R = mybir.dt.float32r
BF16 = mybir.dt.bfloat16

N_HEADS = 16
N_KV_HEADS = 4
DIM = 2048
HD = 128
B = 4
S_FULL = 2048
TP = 4            # tensor-parallel over kv-head groups
DP = 2            # data-parallel over batch
BPC = B // DP     # batches per core
HQ = N_HEADS // TP  # q heads per core
DK = DIM // 128   # contraction tiles over model dim
SCALE = float(1.0 / np.sqrt(HD))

_NC_CACHE = {}


def build_nc(s):
    sc_n = s // 512   # 512-wide s/q chunks
    kt_n = s // 128   # 128-wide key tiles
    st_n = s // 128   # 128-wide s tiles
    kg_n = kt_n // 2  # score groups (2 key tiles each)

    nc = bacc.Bacc("TRN2", target_bir_lowering=False, debug=False)
    xt = nc.dram_tensor("xt", [BPC, DIM, s], BF16, kind="ExternalInput")
    cost = nc.dram_tensor("cost", [BPC, 64, s], BF16, kind="ExternalInput")
    sint = nc.dram_tensor("sint", [BPC, 64, s], BF16, kind="ExternalInput")
    wqt = nc.dram_tensor("wqt", [DIM, HQ * HD], BF16, kind="ExternalInput")
    wkt = nc.dram_tensor("wkt", [DIM, HD], BF16, kind="ExternalInput")
    wvt = nc.dram_tensor("wvt", [DIM, HD], BF16, kind="ExternalInput")
    wot = nc.dram_tensor("wot", [HQ * HD, DIM], BF16, kind="ExternalInput")
    outp = nc.dram_tensor("outp", [BPC, s, DIM], F32, kind="ExternalOutput")

    wqt_v = wqt.rearrange("(dk p) c -> p dk c", p=128)   # [128, DK, 512]
    wkt_v = wkt.rearrange("(dk p) c -> p dk c", p=128)   # [128, DK, 128]
    wvt_v = wvt.rearrange("(dk p) c -> p dk c", p=128)
    wot_v = wot.rearrange("(h p) c -> p h c", p=128)     # [128, HQ, DIM]
    xt_v = xt.rearrange("b (t dkl p) s -> b t p dkl s", dkl=4, p=128)

    with ExitStack() as ctx:
        ctx.enter_context(
            nc.allow_low_precision(reason="bf16 matmul pipeline by design")
        )
        tc = ctx.enter_context(tile.TileContext(nc))

        singles = ctx.enter_context(tc.tile_pool(name="singles", bufs=1))
        qt_pool = ctx.enter_context(tc.tile_pool(name="qt", bufs=1))
        kt_pool = ctx.enter_context(tc.tile_pool(name="ktp", bufs=1))
        v_pool = ctx.enter_context(tc.tile_pool(name="vp", bufs=1))
        e_pool = ctx.enter_context(tc.tile_pool(name="ep", bufs=1))
        ot_pool = ctx.enter_context(tc.tile_pool(name="otp", bufs=1))
        cs_pool = ctx.enter_context(tc.tile_pool(name="csp", bufs=2))
        xt_pool = ctx.enter_context(tc.tile_pool(name="xtp", bufs=12))
        tmp_pool = ctx.enter_context(tc.tile_pool(name="tmp", bufs=2))
        vt_pool = ctx.enter_context(tc.tile_pool(name="vtp", bufs=2))
        csum_pool = ctx.enter_context(tc.tile_pool(name="csum", bufs=2))
        rcp_pool = ctx.enter_context(tc.tile_pool(name="rcp", bufs=2))
        av_pool = ctx.enter_context(tc.tile_pool(name="avp", bufs=2))
        orow_pool = ctx.enter_context(tc.tile_pool(name="orow", bufs=3))

        # One unified PSUM ring: 4 x [128,2,512] slots (all 8 banks). The
        # long-held tiles (kvps in phase A, avsm in phase B) simply make the
        # other allocations rotate over the remaining 3 slots, which gives
        # the score tiles exp(g-3) slack instead of exp(g-2) — enough that
        # the PE never waits on the ACT exp in steady state.
        psum = ctx.enter_context(tc.tile_pool(name="psum", bufs=4, space="PSUM"))

        def ps_tile(name):
            return psum.tile([128, 2, 512], F32, tag="ps", name=name)

        av_tile = ps_tile

        # ---- weights / constants (resident) ----
        # Emission order matters for the Sync DMA-dispatch queue: k/v and the
        # first wq chunk go first so the first projection matmuls can start
        # within a few microseconds.
        wk_sb = singles.tile([128, DK, HD], BF16)
        wv_sb = singles.tile([128, DK, HD], BF16)
        wq_sb = singles.tile([128, DK, HQ * HD], BF16)
        nc.sync.dma_start(out=wq_sb[:, 0:4, :], in_=wqt_v[:, 0:4, :])

        # first chunk's x tiles interleave with the bulk wq load so the
        # head-major dk sweep never outruns the weight transfers. x tiles are
        # loaded with per-dk 2D DMAs: one consolidated 3D dispatch costs
        # 2-5.5us of descriptor generation on the Sync queue and stalls the
        # prefetch pipeline, while 2D dispatches are ~0.65us each.
        def emit_chunk_loads(b, sc, interleave_w=False):
            ss = slice(sc * 512, (sc + 1) * 512)
            extra = [
                lambda: nc.sync.dma_start(out=wk_sb, in_=wkt_v),
                lambda: nc.sync.dma_start(out=wv_sb, in_=wvt_v),
                lambda: nc.sync.dma_start(
                    out=wq_sb[:, 4:8, :], in_=wqt_v[:, 4:8, :]
                ),
                lambda: nc.sync.dma_start(
                    out=wq_sb[:, 8:12, :], in_=wqt_v[:, 8:12, :]
                ),
                lambda: nc.sync.dma_start(
                    out=wq_sb[:, 12:16, :], in_=wqt_v[:, 12:16, :]
                ),
            ]
            tiles = []
            for t in range(4):
                xtile = xt_pool.tile([128, 4, 512], BF16, tag="xt", name=f"x{b}{sc}{t}")
                for dkl in range(4):
                    dk = 4 * t + dkl
                    nc.sync.dma_start(
                        out=xtile[:, dkl, :],
                        in_=xt[b, dk * 128:(dk + 1) * 128, ss],
                    )
                    if interleave_w and extra:
                        extra.pop(0)()
                tiles.append(xtile)
            return tiles

        def emit_cs_loads(b):
            cos2 = cs_pool.tile([128, s], BF16, tag="cs", name=f"cos{b}")
            sin2 = cs_pool.tile([128, s], BF16, tag="cs", name=f"sin{b}")
            for half in range(2):
                nc.sync.dma_start(out=cos2[64 * half:64 * (half + 1), :], in_=cost[b])
                nc.sync.dma_start(out=sin2[64 * half:64 * (half + 1), :], in_=sint[b])
            return cos2, sin2

        pending_cs = emit_cs_loads(0)
        pending_chunks = [emit_chunk_loads(0, 0, interleave_w=True)]

        wo_sb = singles.tile([128, HQ, DIM], BF16)
        wo_loaded = [False]

        def load_wo():
            if not wo_loaded[0]:
                nc.sync.dma_start(out=wo_sb, in_=wot_v)
                wo_loaded[0] = True

        ones32_bf = singles.tile([128, 32], BF16)
        nc.vector.memset(ones32_bf, 1.0)
        sel4_f = singles.tile([128, 128], F32)
        nc.vector.memset(sel4_f, 0.0)
        for j in range(4):
            nc.vector.memset(sel4_f[32 * j:32 * j + 1, :], 1.0)
        sel4 = singles.tile([128, 128], BF16)
        nc.vector.tensor_copy(sel4, sel4_f)
        ident = singles.tile([128, 128], F32)
        make_identity(nc, ident)

        copy_flip = [0]

        def copy_any(dst, src):
            # alternate psum->sbuf copies between ScalarE and VectorE
            if copy_flip[0] % 2 == 0:
                nc.scalar.copy(dst, src)
            else:
                nc.vector.tensor_copy(dst, src)
            copy_flip[0] += 1

        for b in range(BPC):
            # ---- phase A: projections + rope (head-major) ----
            cos2, sin2 = pending_cs
            qt = qt_pool.tile([128, HQ, s], BF16)
            kt = kt_pool.tile([128, s], BF16)
            vsb = v_pool.tile([128, st_n, HD], BF16)

            next_load = sc_n - len(pending_chunks) * sc_n + len(pending_chunks)
            next_load = len(pending_chunks)
            for sc in range(sc_n):
                ss = slice(sc * 512, (sc + 1) * 512)
                xtiles = pending_chunks.pop(0)
                while len(pending_chunks) < 2 and next_load < sc_n:
                    pending_chunks.append(emit_chunk_loads(b, next_load))
                    next_load += 1

                qps = [ps_tile(f"qps{i}") for i in range(2)]
                kvps = av_tile("kvps")

                def mm_contract(dst, wsb, lo, hi):
                    for dk in range(DK):
                        nc.tensor.matmul(
                            dst,
                            wsb[:, dk, lo:hi],
                            xtiles[dk // 4][:, dk % 4, :],
                            start=(dk == 0),
                            stop=(dk == DK - 1),
                        )

                # rope: r' = qr*cos - qi*sin ; i' = qr*sin + qi*cos
                # P1 = [qr;qi] * [cos;cos], P2 = [qr;qi] * [sin;sin]
                # r' = P1[top] - P2[bot] ; i' = P2[top] + P1[bot]
                def rope(src_ps, dst_r, dst_i):
                    # p1 = src*cos (SBUF); then src *= sin in place (PSUM) —
                    # two SBUF TT inputs must share a base partition, but a
                    # PSUM input may sit at any base, so the sin product
                    # stays in the source psum.
                    p1 = tmp_pool.tile([128, 512], F32, tag="tmp")
                    nc.vector.tensor_mul(p1, src_ps, cos2[:, ss])
                    nc.vector.tensor_mul(src_ps, src_ps, sin2[:, ss])
                    nc.vector.tensor_sub(dst_r, p1[0:64, :], src_ps[64:128, :])
                    nc.vector.tensor_add(dst_i, src_ps[0:64, :], p1[64:128, :])

                # head order q0,q1,k,q2,q3,v: the next chunk's first matmuls
                # reuse qps0's PSUM slot, so its ropes must clear the DVE
                # queue early; k's rope (needed only by phase B) goes between
                # the two q pairs instead of after them.
                for m in (0, 1):
                    dst = qps[0][:, m, :]
                    mm_contract(dst, wq_sb, m * HD, (m + 1) * HD)
                    rope(dst, qt[0:64, m, ss], qt[64:128, m, ss])

                mm_contract(kvps[:, 0, :], wk_sb, 0, HD)
                rope(kvps[:, 0, :], kt[0:64, ss], kt[64:128, ss])

                for m in (2, 3):
                    dst = qps[1][:, m - 2, :]
                    mm_contract(dst, wq_sb, m * HD, (m + 1) * HD)
                    rope(dst, qt[0:64, m, ss], qt[64:128, m, ss])

                mm_contract(kvps[:, 1, :], wv_sb, 0, HD)
                # V: copy psum -> sbuf, transpose 128x128 blocks back into the
                # same psum slice, copy out as [s, d] bf16 (ACT-only copies —
                # DVE is busy with the ropes)
                vt_sb = vt_pool.tile([128, 512], F32, tag="vt")
                nc.scalar.copy(vt_sb, kvps[:, 1, :])
                for i in range(4):
                    vtr = kvps[:, 1, i * 128:(i + 1) * 128]
                    nc.tensor.transpose(vtr, vt_sb[:, i * 128:(i + 1) * 128], ident)
                    nc.scalar.copy(vsb[:, sc * 4 + i, :], vtr)

            # ---- phase B: attention, one flat pipeline over score groups ----
            # Global group g = 8*i + kg over all 16 (h,qc) iterations i.
            # Stage g emits: scores(g)+exp(g); AV+colsums for g-2 (two groups
            # late, crossing (h,qc) boundaries so the PE queue never drains on
            # the ACT exp); finalize part1 (copies) when g-2 closes an (h,qc);
            # finalize part2 (bcast+rcp+mul) one stage later so the DVE copies
            # are hidden behind a full group of matmuls.
            outT = ot_pool.tile([128, HQ, s], BF16)
            n_it = HQ * sc_n
            G = n_it * kg_n
            e_ts = {}
            avsms = {}
            csums = {}
            av_sbs = {}

            def hq_of(i):
                return i // sc_n, slice((i % sc_n) * 512, (i % sc_n + 1) * 512)

            for g in range(G + 3):
                if g < G:
                    i, kg = divmod(g, kg_n)
                    h, qs = hq_of(i)
                    if kg == 0:
                        e_ts[i] = e_pool.tile(
                            [128, kt_n, 512], BF16, tag="et", bufs=2, name=f"et{i}"
                        )
                    sc_ps = ps_tile("scps")
                    for j in range(2):
                        ktile = 2 * kg + j
                        nc.tensor.matmul(
                            sc_ps[:, j, :],
                            kt[:, ktile * 128:(ktile + 1) * 128],
                            qt[:, h, qs],
                            start=True,
                            stop=True,
                        )
                    nc.scalar.activation(
                        out=e_ts[i][:, 2 * kg:2 * kg + 2, :],
                        in_=sc_ps,
                        func=mybir.ActivationFunctionType.Exp,
                        scale=SCALE,
                    )
                if 0 <= g - 2 < G:
                    j2, kg2 = divmod(g - 2, kg_n)
                    e_t = e_ts[j2]
                    if kg2 == 0:
                        avsms[j2] = av_tile(f"avsm{j2}")
                    avsm = avsms[j2]
                    for jj in range(2):
                        ktile = 2 * kg2 + jj
                        nc.tensor.matmul(
                            avsm[:, 0, :],
                            vsb[:, ktile, :],
                            e_t[:, ktile, :],
                            start=(ktile == 0),
                            stop=(ktile == kt_n - 1),
                        )
                    if kg2 % 2 == 1:
                        i4 = kg2 // 2
                        # 4 adjacent col-tiled ones-matmuls (pack concurrently)
                        for cj in range(4):
                            ktile = 4 * i4 + cj
                            nc.tensor.matmul(
                                avsm[32 * cj:32 * (cj + 1), 1, :],
                                ones32_bf,
                                e_t[:, ktile, :],
                                start=(i4 == 0),
                                stop=(i4 == kt_n // 4 - 1),
                                tile_position=(0, 32 * cj),
                            )
                    if kg2 == kg_n - 1:
                        # finalize part1: drain avsm to SBUF (DVE)
                        csums[j2] = csum_pool.tile(
                            [128, 512], BF16, tag="csum", name=f"cs{j2}"
                        )
                        nc.vector.tensor_copy(csums[j2], avsm[:, 1, :])
                        av_sbs[j2] = av_pool.tile(
                            [128, 512], BF16, tag="avsb", name=f"avsb{j2}"
                        )
                        nc.vector.tensor_copy(av_sbs[j2], avsm[:, 0, :])
                if g - 3 >= 0 and (g - 3) % kg_n == kg_n - 1:
                    # finalize part2: denominator broadcast + normalize
                    j3 = (g - 3) // kg_n
                    h3, qs3 = hq_of(j3)
                    avsm = avsms.pop(j3)
                    nc.tensor.matmul(
                        avsm[:, 1, :], sel4, csums.pop(j3), start=True, stop=True
                    )
                    rcp = rcp_pool.tile([128, 512], F32, tag="rcp")
                    nc.vector.reciprocal_approx_fast(out=rcp, in_=avsm[:, 1, :])
                    nc.vector.tensor_mul(outT[:, h3, qs3], av_sbs.pop(j3), rcp)

            # prefetch next batch's first chunk + cos/sin during phase C
            if b + 1 < BPC:
                pending_chunk = emit_chunk_loads(b + 1, 0)
                pending_cs = emit_cs_loads(b + 1)

            # ---- phase C: output projection ----
            # ops tiles alternate between the "pair" and "av" PSUM tags for an
            # effective 4-slot rotation, and each group's drain is split
            # between ScalarE and VectorE so the slot frees in ~0.7us — both
            # needed to keep the PE from waiting on psum drains.
            load_wo()
            cgrp = [0]
            for scb in range(st_n):
                sb_ = slice(scb * 128, (scb + 1) * 128)
                for dc in range(0, DIM // 512, 2):
                    if cgrp[0] % 2 == 0:
                        ops_ = ps_tile("ops")
                    else:
                        ops_ = av_tile("ops")
                    cgrp[0] += 1
                    for jj in range(2):
                        for h2 in range(HQ):
                            nc.tensor.matmul(
                                ops_[:, jj, :],
                                outT[:, h2, sb_],
                                wo_sb[:, h2, (dc + jj) * 512:(dc + jj + 1) * 512],
                                start=(h2 == 0),
                                stop=(h2 == HQ - 1),
                            )
                    orow = orow_pool.tile([128, 2, 512], F32, tag="orow")
                    nc.scalar.copy(orow[:, 0, :], ops_[:, 0, :])
                    nc.vector.tensor_copy(orow[:, 1, :], ops_[:, 1, :])
                    nc.sync.dma_start(
                        out=outp[b, sb_, dc * 512:(dc + 2) * 512],
                        in_=orow.rearrange("p a b -> p (a b)"),
                    )

    nc.compile()
    return nc


_PERM = np.concatenate([np.arange(0, HD, 2), np.arange(1, HD, 2)])


def _prep_inputs(x, freqs, wq, wk, wv, wo, s):
    """Build the 8 per-core input maps."""
    in_maps = []
    xt_dp = []
    cos_dp = []
    sin_dp = []
    for dp in range(DP):
        bs = slice(dp * BPC, (dp + 1) * BPC)
        xt_dp.append(
            np.ascontiguousarray(x[bs].transpose(0, 2, 1)).astype(ml_dtypes.bfloat16)
        )
        cos_dp.append(
            np.ascontiguousarray(np.cos(freqs[bs]).transpose(0, 2, 1)).astype(
                ml_dtypes.bfloat16
            )
        )
        sin_dp.append(
            np.ascontiguousarray(np.sin(freqs[bs]).transpose(0, 2, 1)).astype(
                ml_dtypes.bfloat16
            )
        )
    for core in range(8):
        g = core % TP
        dp = core // TP
        wq_g = wq[g * HQ * HD:(g + 1) * HQ * HD]  # [512, DIM]
        wq_p = wq_g.reshape(HQ, HD, DIM)[:, _PERM, :].reshape(HQ * HD, DIM)
        wk_g = wk[g * HD:(g + 1) * HD][_PERM]      # [128, DIM]
        wv_g = wv[g * HD:(g + 1) * HD]             # [128, DIM]
        wo_g = wo[:, g * HQ * HD:(g + 1) * HQ * HD]  # [DIM, 512]
        in_maps.append(
            {
                "xt": xt_dp[dp],
                "cost": cos_dp[dp],
                "sint": sin_dp[dp],
                "wqt": np.ascontiguousarray(wq_p.T).astype(ml_dtypes.bfloat16),
                "wkt": np.ascontiguousarray(wk_g.T).astype(ml_dtypes.bfloat16),
                "wvt": np.ascontiguousarray(wv_g.T).astype(ml_dtypes.bfloat16),
                "wot": np.ascontiguousarray(wo_g.T).astype(ml_dtypes.bfloat16),
            }
        )
    return in_maps


_LAST = {}


def _run(x, freqs, wq, wk, wv, wo, s):
    x = np.asarray(x, dtype=np.float32)
    freqs = np.asarray(freqs, dtype=np.float32)
    wq = np.asarray(wq, dtype=np.float32)
    wk = np.asarray(wk, dtype=np.float32)
    wv = np.asarray(wv, dtype=np.float32)
    wo = np.asarray(wo, dtype=np.float32)

    if s not in _NC_CACHE:
        _NC_CACHE[s] = build_nc(s)
    nc = _NC_CACHE[s]
    in_maps = _prep_inputs(x, freqs, wq, wk, wv, wo, s)
    res = run_bass_kernel_spmd(nc, in_maps, core_ids=list(range(8)))
    _LAST["nc"] = nc
    _LAST["in_maps"] = in_maps

    out = np.empty((B, s, DIM), dtype=np.float32)
    for dp in range(DP):
        acc = res.results[dp * TP]["outp"].copy()
        for g in range(1, TP):
            acc += res.results[dp * TP + g]["outp"]
        out[dp * BPC:(dp + 1) * BPC] = acc
    return out


def kernel(x, freqs, wq, wk, wv, wo):
    return _run(x, freqs, wq, wk, wv, wo, S_FULL)


# revision 21
# speedup vs baseline: 1.2345x; 1.2345x over previous
"""Trainium2 Bass kernel for nn_Attention (dense transformer attention block).

Computation (per reference):
  q = x @ wq.T; k = x @ wk.T; v = x @ wv.T       (GQA: 16 q heads, 4 kv heads)
  rope(q, k) with cos/sin from freqs (interleaved complex pairs)
  non-causal SDPA with softmax over keys, scale 1/sqrt(128)
  out = (probs @ v reshaped) @ wo.T

Sharding (8 cores): tensor-parallel over the 4 kv-head groups (TP=4; each
core gets 4 q heads + 1 kv head, wq/wk/wv column-sharded, wo row-sharded)
x data-parallel over batch (DP=2; 2 batches per core). Each core computes a
partial output [2, S, DIM]; the host sums the 4 TP partials per batch pair.

Device layout notes:
 - x is passed transposed per batch: xt[b] = x[b].T  [DIM, S] (bf16) so that
   projections contract over DIM on the partition axis.
 - All matmul operands are bf16 (fp32 PSUM accumulation): enables fast
   weight load and halves HBM traffic; measured end-to-end error ~7e-3.
 - Q^T/K^T are computed in [head_dim, S] layout; per head the 128
   head-dim rows are permuted to [evens(64) | odds(64)] (done by permuting
   wq/wk rows on the host) so RoPE pairs are partition-contiguous halves.
   RoPE runs per head as soon as that head's accumulation finishes
   (head-major matmul order) so PSUM slots recycle without stalling PE.
 - scores are computed transposed: S^T[k, q] = K^T.T @ Q^T per 128k x 512q
   tile; exp via ACT (scale folded in). Phase B is software-pipelined: the
   AV / denominator matmuls for score-group kg are emitted one group late
   so the in-order PE queue never stalls on the ACT exp.
 - softmax denominators: 4 col-tiled ones-matmuls (M=32 broadcast rows,
   emitted adjacently so they pack into the PE array concurrently)
   accumulate per-column partial sums; a selection-matrix matmul sums the
   4 partials and broadcasts to 128 partitions; reciprocal on DVE; the AV
   output is scaled by it.
 - PSUM plan (8 banks): tag "pair" = 3 x [128,2,512] rotating slots
   (projection q pairs / score groups / phase C), tag "av" = 1 dedicated
   slot (projection k+v / AV+colsum accumulator).
 - AV output is produced in [head_dim, S] layout which directly feeds the
   wo matmul as lhsT.
"""

import numpy as np
from contextlib import ExitStack

import ml_dtypes

import concourse.bacc as bacc
import concourse.tile as tile
from concourse import mybir
from concourse.bass_utils import run_bass_kernel_spmd
from concourse.masks import make_identity

F32 = mybir.dt.float32
F32R = mybir.dt.float32r
BF16 = mybir.dt.bfloat16

N_HEADS = 16
N_KV_HEADS = 4
DIM = 2048
HD = 128
B = 4
S_FULL = 2048
TP = 4            # tensor-parallel over kv-head groups
DP = 2            # data-parallel over batch
BPC = B // DP     # batches per core
HQ = N_HEADS // TP  # q heads per core
DK = DIM // 128   # contraction tiles over model dim
SCALE = float(1.0 / np.sqrt(HD))

_NC_CACHE = {}


def build_nc(s):
    sc_n = s // 512   # 512-wide s/q chunks
    kt_n = s // 128   # 128-wide key tiles
    st_n = s // 128   # 128-wide s tiles
    kg_n = kt_n // 2  # score groups (2 key tiles each)

    nc = bacc.Bacc("TRN2", target_bir_lowering=False, debug=False)
    xt = nc.dram_tensor("xt", [BPC, DIM, s], BF16, kind="ExternalInput")
    cost = nc.dram_tensor("cost", [BPC, 64, s], BF16, kind="ExternalInput")
    sint = nc.dram_tensor("sint", [BPC, 64, s], BF16, kind="ExternalInput")
    wqt = nc.dram_tensor("wqt", [DIM, HQ * HD], BF16, kind="ExternalInput")
    wkt = nc.dram_tensor("wkt", [DIM, HD], BF16, kind="ExternalInput")
    wvt = nc.dram_tensor("wvt", [DIM, HD], BF16, kind="ExternalInput")
    wot = nc.dram_tensor("wot", [HQ * HD, DIM], BF16, kind="ExternalInput")
    outp = nc.dram_tensor("outp", [BPC, s, DIM], F32, kind="ExternalOutput")

    wqt_v = wqt.rearrange("(dk p) c -> p dk c", p=128)   # [128, DK, 512]
    wkt_v = wkt.rearrange("(dk p) c -> p dk c", p=128)   # [128, DK, 128]
    wvt_v = wvt.rearrange("(dk p) c -> p dk c", p=128)
    wot_v = wot.rearrange("(h p) c -> p h c", p=128)     # [128, HQ, DIM]
    xt_v = xt.rearrange("b (t dkl p) s -> b t p dkl s", dkl=4, p=128)

    with ExitStack() as ctx:
        ctx.enter_context(
            nc.allow_low_precision(reason="bf16 matmul pipeline by design")
        )
        tc = ctx.enter_context(tile.TileContext(nc))

        singles = ctx.enter_context(tc.tile_pool(name="singles", bufs=1))
        qt_pool = ctx.enter_context(tc.tile_pool(name="qt", bufs=1))
        kt_pool = ctx.enter_context(tc.tile_pool(name="ktp", bufs=1))
        v_pool = ctx.enter_context(tc.tile_pool(name="vp", bufs=1))
        e_pool = ctx.enter_context(tc.tile_pool(name="ep", bufs=1))
        ot_pool = ctx.enter_context(tc.tile_pool(name="otp", bufs=1))
        cs_pool = ctx.enter_context(tc.tile_pool(name="csp", bufs=2))
        xt_pool = ctx.enter_context(tc.tile_pool(name="xtp", bufs=12))
        tmp_pool = ctx.enter_context(tc.tile_pool(name="tmp", bufs=2))
        vt_pool = ctx.enter_context(tc.tile_pool(name="vtp", bufs=2))
        csum_pool = ctx.enter_context(tc.tile_pool(name="csum", bufs=2))
        rcp_pool = ctx.enter_context(tc.tile_pool(name="rcp", bufs=2))
        av_pool = ctx.enter_context(tc.tile_pool(name="avp", bufs=2))
        orow_pool = ctx.enter_context(tc.tile_pool(name="orow", bufs=3))

        # One unified PSUM ring: 4 x [128,2,512] slots (all 8 banks). The
        # long-held tiles (kvps in phase A, avsm in phase B) simply make the
        # other allocations rotate over the remaining 3 slots, which gives
        # the score tiles exp(g-3) slack instead of exp(g-2) — enough that
        # the PE never waits on the ACT exp in steady state.
        psum = ctx.enter_context(tc.tile_pool(name="psum", bufs=4, space="PSUM"))

        def ps_tile(name):
            return psum.tile([128, 2, 512], F32, tag="ps", name=name)

        av_tile = ps_tile

        # ---- weights / constants (resident) ----
        # Emission order matters for the Sync DMA-dispatch queue: k/v and the
        # first wq chunk go first so the first projection matmuls can start
        # within a few microseconds.
        wk_sb = singles.tile([128, DK, HD], BF16)
        wv_sb = singles.tile([128, DK, HD], BF16)
        wq_sb = singles.tile([128, DK, HQ * HD], BF16)
        nc.sync.dma_start(out=wq_sb[:, 0:4, :], in_=wqt_v[:, 0:4, :])

        # first chunk's x tiles interleave with the bulk wq load so the
        # head-major dk sweep never outruns the weight transfers. x tiles are
        # loaded with per-dk 2D DMAs: one consolidated 3D dispatch costs
        # 2-5.5us of descriptor generation on the Sync queue and stalls the
        # prefetch pipeline, while 2D dispatches are ~0.65us each.
        def emit_chunk_loads(b, sc, interleave_w=False):
            ss = slice(sc * 512, (sc + 1) * 512)
            extra = [
                lambda: nc.sync.dma_start(out=wk_sb, in_=wkt_v),
                lambda: nc.sync.dma_start(out=wv_sb, in_=wvt_v),
                lambda: nc.sync.dma_start(
                    out=wq_sb[:, 4:8, :], in_=wqt_v[:, 4:8, :]
                ),
                lambda: nc.sync.dma_start(
                    out=wq_sb[:, 8:12, :], in_=wqt_v[:, 8:12, :]
                ),
                lambda: nc.sync.dma_start(
                    out=wq_sb[:, 12:16, :], in_=wqt_v[:, 12:16, :]
                ),
            ]
            tiles = []
            for t in range(4):
                xtile = xt_pool.tile([128, 4, 512], BF16, tag="xt", name=f"x{b}{sc}{t}")
                for dkl in range(4):
                    dk = 4 * t + dkl
                    nc.sync.dma_start(
                        out=xtile[:, dkl, :],
                        in_=xt[b, dk * 128:(dk + 1) * 128, ss],
                    )
                    if interleave_w and extra:
                        extra.pop(0)()
                tiles.append(xtile)
            return tiles

        def emit_cs_loads(b):
            cos2 = cs_pool.tile([128, s], BF16, tag="cs", name=f"cos{b}")
            sin2 = cs_pool.tile([128, s], BF16, tag="cs", name=f"sin{b}")
            for half in range(2):
                nc.sync.dma_start(out=cos2[64 * half:64 * (half + 1), :], in_=cost[b])
                nc.sync.dma_start(out=sin2[64 * half:64 * (half + 1), :], in_=sint[b])
            return cos2, sin2

        pending_cs = emit_cs_loads(0)
        pending_chunks = [emit_chunk_loads(0, 0, interleave_w=True)]

        wo_sb = singles.tile([128, HQ, DIM], BF16)
        wo_loaded = [False]

        def load_wo():
            if not wo_loaded[0]:
                nc.sync.dma_start(out=wo_sb, in_=wot_v)
                wo_loaded[0] = True

        ones32_bf = singles.tile([128, 32], BF16)
        nc.vector.memset(ones32_bf, 1.0)
        sel4_f = singles.tile([128, 128], F32)
        nc.vector.memset(sel4_f, 0.0)
        for j in range(4):
            nc.vector.memset(sel4_f[32 * j:32 * j + 1, :], 1.0)
        sel4 = singles.tile([128, 128], BF16)
        nc.vector.tensor_copy(sel4, sel4_f)
        ident = singles.tile([128, 128], F32)
        make_identity(nc, ident)

        copy_flip = [0]

        def copy_any(dst, src):
            # alternate psum->sbuf copies between ScalarE and VectorE
            if copy_flip[0] % 2 == 0:
                nc.scalar.copy(dst, src)
            else:
                nc.vector.tensor_copy(dst, src)
            copy_flip[0] += 1

        for b in range(BPC):
            # ---- phase A: projections + rope (head-major) ----
            cos2, sin2 = pending_cs
            qt = qt_pool.tile([128, HQ, s], BF16)
            kt = kt_pool.tile([128, s], BF16)
            vsb = v_pool.tile([128, st_n, HD], BF16)

            next_load = len(pending_chunks)
            for sc in range(sc_n):
                ss = slice(sc * 512, (sc + 1) * 512)
                xtiles = pending_chunks.pop(0)
                while len(pending_chunks) < 2 and next_load < sc_n:
                    pending_chunks.append(emit_chunk_loads(b, next_load))
                    next_load += 1

                qps = [ps_tile(f"qps{i}") for i in range(2)]
                kvps = av_tile("kvps")

                def mm_contract(dst, wsb, lo, hi):
                    for dk in range(DK):
                        nc.tensor.matmul(
                            dst,
                            wsb[:, dk, lo:hi],
                            xtiles[dk // 4][:, dk % 4, :],
                            start=(dk == 0),
                            stop=(dk == DK - 1),
                        )

                # rope: r' = qr*cos - qi*sin ; i' = qr*sin + qi*cos
                # P1 = [qr;qi] * [cos;cos], P2 = [qr;qi] * [sin;sin]
                # r' = P1[top] - P2[bot] ; i' = P2[top] + P1[bot]
                def rope(src_ps, dst_r, dst_i):
                    # p1 = src*cos (SBUF); then src *= sin in place (PSUM) —
                    # two SBUF TT inputs must share a base partition, but a
                    # PSUM input may sit at any base, so the sin product
                    # stays in the source psum.
                    p1 = tmp_pool.tile([128, 512], F32, tag="tmp")
                    nc.vector.tensor_mul(p1, src_ps, cos2[:, ss])
                    nc.vector.tensor_mul(src_ps, src_ps, sin2[:, ss])
                    nc.vector.tensor_sub(dst_r, p1[0:64, :], src_ps[64:128, :])
                    nc.vector.tensor_add(dst_i, src_ps[0:64, :], p1[64:128, :])

                # head order q0,q1,k,q2,q3,v: the next chunk's first matmuls
                # reuse qps0's PSUM slot, so its ropes must clear the DVE
                # queue early; k's rope (needed only by phase B) goes between
                # the two q pairs instead of after them.
                for m in (0, 1):
                    dst = qps[0][:, m, :]
                    mm_contract(dst, wq_sb, m * HD, (m + 1) * HD)
                    rope(dst, qt[0:64, m, ss], qt[64:128, m, ss])

                mm_contract(kvps[:, 0, :], wk_sb, 0, HD)
                rope(kvps[:, 0, :], kt[0:64, ss], kt[64:128, ss])

                for m in (2, 3):
                    dst = qps[1][:, m - 2, :]
                    mm_contract(dst, wq_sb, m * HD, (m + 1) * HD)
                    rope(dst, qt[0:64, m, ss], qt[64:128, m, ss])

                mm_contract(kvps[:, 1, :], wv_sb, 0, HD)
                # V: copy psum -> sbuf, transpose 128x128 blocks back into the
                # same psum slice, copy out as [s, d] bf16 (ACT-only copies —
                # DVE is busy with the ropes)
                vt_sb = vt_pool.tile([128, 512], F32, tag="vt")
                nc.scalar.copy(vt_sb, kvps[:, 1, :])
                for i in range(4):
                    vtr = kvps[:, 1, i * 128:(i + 1) * 128]
                    nc.tensor.transpose(vtr, vt_sb[:, i * 128:(i + 1) * 128], ident)
                    nc.scalar.copy(vsb[:, sc * 4 + i, :], vtr)

            # ---- phase B: attention, one flat pipeline over score groups ----
            # Global group g = 8*i + kg over all 16 (h,qc) iterations i.
            # Stage g emits: scores(g)+exp(g); AV+colsums for g-2 (two groups
            # late, crossing (h,qc) boundaries so the PE queue never drains on
            # the ACT exp); finalize part1 (copies) when g-2 closes an (h,qc);
            # finalize part2 (bcast+rcp+mul) one stage later so the DVE copies
            # are hidden behind a full group of matmuls.
            outT = ot_pool.tile([128, HQ, s], BF16)
            n_it = HQ * sc_n
            G = n_it * kg_n
            e_ts = {}
            avsms = {}
            csums = {}
            av_sbs = {}

            def hq_of(i):
                return i // sc_n, slice((i % sc_n) * 512, (i % sc_n + 1) * 512)

            for g in range(G + 3):
                if g < G:
                    i, kg = divmod(g, kg_n)
                    h, qs = hq_of(i)
                    if kg == 0:
                        e_ts[i] = e_pool.tile(
                            [128, kt_n, 512], BF16, tag="et", bufs=2, name=f"et{i}"
                        )
                    sc_ps = ps_tile("scps")
                    for j in range(2):
                        ktile = 2 * kg + j
                        nc.tensor.matmul(
                            sc_ps[:, j, :],
                            kt[:, ktile * 128:(ktile + 1) * 128],
                            qt[:, h, qs],
                            start=True,
                            stop=True,
                        )
                    nc.scalar.activation(
                        out=e_ts[i][:, 2 * kg:2 * kg + 2, :],
                        in_=sc_ps,
                        func=mybir.ActivationFunctionType.Exp,
                        scale=SCALE,
                    )
                if 0 <= g - 2 < G:
                    j2, kg2 = divmod(g - 2, kg_n)
                    e_t = e_ts[j2]
                    if kg2 == 0:
                        avsms[j2] = av_tile(f"avsm{j2}")
                    avsm = avsms[j2]
                    for jj in range(2):
                        ktile = 2 * kg2 + jj
                        nc.tensor.matmul(
                            avsm[:, 0, :],
                            vsb[:, ktile, :],
                            e_t[:, ktile, :],
                            start=(ktile == 0),
                            stop=(ktile == kt_n - 1),
                        )
                    if kg2 % 2 == 1:
                        i4 = kg2 // 2
                        # 4 adjacent col-tiled ones-matmuls (pack concurrently)
                        for cj in range(4):
                            ktile = 4 * i4 + cj
                            nc.tensor.matmul(
                                avsm[32 * cj:32 * (cj + 1), 1, :],
                                ones32_bf,
                                e_t[:, ktile, :],
                                start=(i4 == 0),
                                stop=(i4 == kt_n // 4 - 1),
                                tile_position=(0, 32 * cj),
                            )
                    if kg2 == kg_n - 1:
                        # finalize part1: drain avsm to SBUF (DVE)
                        csums[j2] = csum_pool.tile(
                            [128, 512], BF16, tag="csum", name=f"cs{j2}"
                        )
                        nc.vector.tensor_copy(csums[j2], avsm[:, 1, :])
                        av_sbs[j2] = av_pool.tile(
                            [128, 512], BF16, tag="avsb", name=f"avsb{j2}"
                        )
                        nc.vector.tensor_copy(av_sbs[j2], avsm[:, 0, :])
                if g - 3 >= 0 and (g - 3) % kg_n == kg_n - 1:
                    # finalize part2: denominator broadcast + normalize
                    j3 = (g - 3) // kg_n
                    h3, qs3 = hq_of(j3)
                    avsm = avsms.pop(j3)
                    nc.tensor.matmul(
                        avsm[:, 1, :], sel4, csums.pop(j3), start=True, stop=True
                    )
                    rcp = rcp_pool.tile([128, 512], F32, tag="rcp")
                    nc.vector.reciprocal_approx_fast(out=rcp, in_=avsm[:, 1, :])
                    nc.vector.tensor_mul(outT[:, h3, qs3], av_sbs.pop(j3), rcp)

            # prefetch next batch's first two chunks + cos/sin ahead of phase
            # C's output DMAs so they aren't queued behind them on Sync
            if b + 1 < BPC:
                pending_cs = emit_cs_loads(b + 1)
                pending_chunks = [
                    emit_chunk_loads(b + 1, 0),
                    emit_chunk_loads(b + 1, 1),
                ]

            # ---- phase C: output projection ----
            # ops tiles alternate between the "pair" and "av" PSUM tags for an
            # effective 4-slot rotation, and each group's drain is split
            # between ScalarE and VectorE so the slot frees in ~0.7us — both
            # needed to keep the PE from waiting on psum drains.
            load_wo()
            cgrp = [0]
            for scb in range(st_n):
                sb_ = slice(scb * 128, (scb + 1) * 128)
                for dc in range(0, DIM // 512, 2):
                    if cgrp[0] % 2 == 0:
                        ops_ = ps_tile("ops")
                    else:
                        ops_ = av_tile("ops")
                    cgrp[0] += 1
                    for jj in range(2):
                        for h2 in range(HQ):
                            nc.tensor.matmul(
                                ops_[:, jj, :],
                                outT[:, h2, sb_],
                                wo_sb[:, h2, (dc + jj) * 512:(dc + jj + 1) * 512],
                                start=(h2 == 0),
                                stop=(h2 == HQ - 1),
                            )
                    orow = orow_pool.tile([128, 2, 512], F32, tag="orow")
                    nc.scalar.copy(orow[:, 0, :], ops_[:, 0, :])
                    nc.vector.tensor_copy(orow[:, 1, :], ops_[:, 1, :])
                    nc.sync.dma_start(
                        out=outp[b, sb_, dc * 512:(dc + 2) * 512],
                        in_=orow.rearrange("p a b -> p (a b)"),
                    )

    nc.compile()
    return nc


_PERM = np.concatenate([np.arange(0, HD, 2), np.arange(1, HD, 2)])


def _prep_inputs(x, freqs, wq, wk, wv, wo, s):
    """Build the 8 per-core input maps."""
    in_maps = []
    xt_dp = []
    cos_dp = []
    sin_dp = []
    for dp in range(DP):
        bs = slice(dp * BPC, (dp + 1) * BPC)
        xt_dp.append(
            np.ascontiguousarray(x[bs].transpose(0, 2, 1)).astype(ml_dtypes.bfloat16)
        )
        cos_dp.append(
            np.ascontiguousarray(np.cos(freqs[bs]).transpose(0, 2, 1)).astype(
                ml_dtypes.bfloat16
            )
        )
        sin_dp.append(
            np.ascontiguousarray(np.sin(freqs[bs]).transpose(0, 2, 1)).astype(
                ml_dtypes.bfloat16
            )
        )
    for core in range(8):
        g = core % TP
        dp = core // TP
        wq_g = wq[g * HQ * HD:(g + 1) * HQ * HD]  # [512, DIM]
        wq_p = wq_g.reshape(HQ, HD, DIM)[:, _PERM, :].reshape(HQ * HD, DIM)
        wk_g = wk[g * HD:(g + 1) * HD][_PERM]      # [128, DIM]
        wv_g = wv[g * HD:(g + 1) * HD]             # [128, DIM]
        wo_g = wo[:, g * HQ * HD:(g + 1) * HQ * HD]  # [DIM, 512]
        in_maps.append(
            {
                "xt": xt_dp[dp],
                "cost": cos_dp[dp],
                "sint": sin_dp[dp],
                "wqt": np.ascontiguousarray(wq_p.T).astype(ml_dtypes.bfloat16),
                "wkt": np.ascontiguousarray(wk_g.T).astype(ml_dtypes.bfloat16),
                "wvt": np.ascontiguousarray(wv_g.T).astype(ml_dtypes.bfloat16),
                "wot": np.ascontiguousarray(wo_g.T).astype(ml_dtypes.bfloat16),
            }
        )
    return in_maps


_LAST = {}


def _run(x, freqs, wq, wk, wv, wo, s):
    x = np.asarray(x, dtype=np.float32)
    freqs = np.asarray(freqs, dtype=np.float32)
    wq = np.asarray(wq, dtype=np.float32)
    wk = np.asarray(wk, dtype=np.float32)
    wv = np.asarray(wv, dtype=np.float32)
    wo = np.asarray(wo, dtype=np.float32)

    if s not in _NC_CACHE:
        _NC_CACHE[s] = build_nc(s)
    nc = _NC_CACHE[s]
    in_maps = _prep_inputs(x, freqs, wq, wk, wv, wo, s)
    res = run_bass_kernel_spmd(nc, in_maps, core_ids=list(range(8)))
    _LAST["nc"] = nc
    _LAST["in_maps"] = in_maps

    out = np.empty((B, s, DIM), dtype=np.float32)
    for dp in range(DP):
        acc = res.results[dp * TP]["outp"].copy()
        for g in range(1, TP):
            acc += res.results[dp * TP + g]["outp"]
        out[dp * BPC:(dp + 1) * BPC] = acc
    return out


def kernel(x, freqs, wq, wk, wv, wo):
    return _run(x, freqs, wq, wk, wv, wo, S_FULL)
